# revision 22
# baseline (speedup 1.0000x reference)
"""ASGC layer (gnn_message_passing) Trainium2 kernel.

Strategy: shard dst nodes 8 ways (graph-parallel per sharding hint). Host does
integer-only index preprocessing (edge bucketing by dst shard/block, slot
assignment, degree counting via bincount); all floating-point math runs on
device:
  - dma_gather fetches feature rows (padded to 512B) from HBM by src index
  - DVE builds norm[src]-weighted one-hot scatter matrices per 64-node block
  - PE matmuls accumulate segment sums into PSUM
  - ACT fuses the norm[dst] scale into the PSUM->SBUF copy
  - DVE/ACT compute the sigmoid gate and final output

src node ids exceed int16 gather-index range, so the padded feature table is
split into lo/hi halves at row 25088 and each block's edges are partitioned
into lo/hi slot groups (statically sized at max-over-cores).
"""

import numpy as np

N = 50000
D = 96
NPAD = 50176  # 392*128
NCORES = 8
SHARD = NPAD // NCORES  # 6272
W = 64  # dst nodes per scatter block (one-hot width)
BLOCKS = SHARD // W  # 98
BPG = 6  # blocks per gather group (bounded: each dma_gather call burns one
# Pool register via to_reg, and the register file is ~48)
NGROUPS = (BLOCKS + BPG - 1) // BPG
NPAIRS = BLOCKS // 2  # 49 [128,96] output tiles per core
SPLIT = 25088  # lo/hi gather table split
OUT_CH = 7  # pairs per output chunk (49 = 7*7)


def _cdiv(a, b):
    return (a + b - 1) // b


def _host_prep(src, dst):
    """Integer-only index preprocessing. Returns static schedule + per-core
    device input arrays."""
    src = np.asarray(src).astype(np.int64)
    dst = np.asarray(dst).astype(np.int64)
    deg = np.bincount(dst, minlength=NPAD).astype(np.int64)
    deg_cl = np.maximum(deg, 1).astype(np.float32)

    core_of_edge = dst // SHARD

    # per-core sorted edge arrays and per-seg counts; seg = block*2 + half
    NSEG = BLOCKS * 2
    per_core = []
    cnt = np.zeros((NCORES, NSEG), dtype=np.int64)
    for c in range(NCORES):
        m = core_of_edge == c
        s_c, d_c = src[m], dst[m]
        blk = (d_c - c * SHARD) // W
        half = (s_c >= SPLIT).astype(np.int64)
        seg = blk * 2 + half
        order = np.argsort(seg, kind="stable")
        s_c, d_c, seg = s_c[order], d_c[order], seg[order]
        cnt[c] = np.bincount(seg, minlength=NSEG)
        per_core.append((s_c, d_c, seg))

    # static slots per seg: max over cores, >= 1
    slots_of_seg = np.maximum(_cdiv(cnt.max(axis=0), 128), 1)  # ceil div
    slots_of_seg = np.maximum(
        (cnt.max(axis=0) + 127) // 128, 1
    ).astype(np.int64)

    groups = [
        list(range(g * BPG, min((g + 1) * BPG, BLOCKS))) for g in range(NGROUPS)
    ]

    # slot order: per group: [lo slots of each block, then hi slots of each block]
    slot_start = np.zeros(NSEG, dtype=np.int64)
    call_info = []  # per group: dict(lo=(slot0, nslots), hi=(...)) in slots
    cursor = 0
    for bs in groups:
        ginfo = {}
        for half in (0, 1):
            first = cursor
            for b in bs:
                seg = 2 * b + half
                slot_start[seg] = cursor
                cursor += int(slots_of_seg[seg])
            ginfo["lo" if half == 0 else "hi"] = (int(first), int(cursor - first))
        call_info.append(ginfo)
    total_slots = int(cursor)

    # gidx column layout: calls in order (g0 lo, g0 hi, g1 lo, ...), each call
    # with nslots*8 int16 columns
    col_cursor = 0
    call_cols = []
    for g in range(NGROUPS):
        lo0, lon = call_info[g]["lo"]
        hi0, hin = call_info[g]["hi"]
        call_cols.append((int(col_cursor), int(col_cursor + lon * 8)))
        col_cursor += (lon + hin) * 8
    gidx_cols = int(col_cursor)

    # per-core arrays
    cores = []
    for c in range(NCORES):
        s_c, d_c, seg = per_core[c]
        ne = len(s_c)
        seg_first = np.searchsorted(seg, np.arange(NSEG))
        rank = np.arange(ne) - seg_first[seg]
        slot = slot_start[seg] + rank // 128  # global slot column
        lane = rank % 128

        dstloc = np.full((128, total_slots), -1.0, dtype=np.float32)
        degsrc = np.ones((128, total_slots), dtype=np.float32)
        dstloc[lane, slot] = (d_c - (dst_base := c * SHARD) - (seg // 2) * W).astype(
            np.float32
        )
        degsrc[lane, slot] = deg_cl[s_c]

        # gather indices: position within call = (slot - call_slot0)*128 + lane
        gidx = np.zeros((128, gidx_cols), dtype=np.int16)
        idx_val = (s_c - (seg % 2) * SPLIT).astype(np.int16)
        # call id = group*2 + half; call slot0/col0 lookup per seg
        seg_g = (seg // 2) // BPG
        seg_half = seg % 2
        call_slot0 = np.zeros(NSEG, dtype=np.int64)
        call_col0 = np.zeros(NSEG, dtype=np.int64)
        for g in range(NGROUPS):
            lo0, lon = call_info[g]["lo"]
            hi0, hin = call_info[g]["hi"]
            c0 = call_cols[g][0]
            for b in groups[g]:
                call_slot0[2 * b] = lo0
                call_col0[2 * b] = c0
                call_slot0[2 * b + 1] = hi0
                call_col0[2 * b + 1] = c0 + lon * 8
        i_call = (slot - call_slot0[seg]) * 128 + lane
        col = call_col0[seg] + i_call // 16
        row = i_call % 16
        for rep in range(8):
            gidx[row + rep * 16, col] = idx_val

        degdst = deg_cl[c * SHARD : (c + 1) * SHARD].reshape(NPAIRS, 128).T

        # pack all f32 metadata into one tensor: one DMA -> one sem wait on
        # consumers (walrus caps sync waits per instruction)
        meta = np.zeros((128, 2 * total_slots + NPAIRS + W + 2 * D), dtype=np.float32)
        meta[:, :total_slots] = dstloc
        meta[:, total_slots : 2 * total_slots] = degsrc
        c0 = 2 * total_slots
        meta[:, c0 : c0 + NPAIRS] = degdst
        meta[:, c0 + NPAIRS : c0 + NPAIRS + W] = np.arange(W, dtype=np.float32)[
            None, :
        ]
        cores.append(dict(gidx=gidx, meta=meta, dstloc=dstloc))

    sgmax = max(
        call_info[g]["lo"][1] + call_info[g]["hi"][1] for g in range(NGROUPS)
    )
    static = dict(
        slots_of_seg=slots_of_seg,
        slot_start=slot_start,
        groups=groups,
        call_info=call_info,
        call_cols=call_cols,
        total_slots=total_slots,
        gidx_cols=gidx_cols,
        sgmax=sgmax,
    )
    return static, cores


def _build_kernel(static):
    import concourse.bacc as bacc
    import concourse.bass as bass
    import concourse.mybir as mybir
    import concourse.tile as tile

    slots_of_seg = static["slots_of_seg"]
    slot_start = static["slot_start"]
    groups = static["groups"]
    call_info = static["call_info"]
    call_cols = static["call_cols"]
    TOT = static["total_slots"]
    GCOLS = static["gidx_cols"]
    SGMAX = max(
        call_info[g]["lo"][1] + call_info[g]["hi"][1] for g in range(NGROUPS)
    )

    f32 = mybir.dt.float32
    bf16 = mybir.dt.bfloat16
    i16 = mybir.dt.int16
    AF = mybir.ActivationFunctionType
    OP = mybir.AluOpType

    # message-passing path dtype (gather tables / gathered tiles / one-hots);
    # PSUM accumulation, degrees, gate and output stay fp32
    import os as _os

    USE_BF16 = _os.environ.get("K_DT", "bf16") == "bf16"
    mdt = bf16 if USE_BF16 else f32

    MCOLS = 2 * TOT + NPAIRS + W + 2 * D

    nc = bacc.Bacc(None, target_bir_lowering=False)
    flo = nc.dram_tensor("flo", [SPLIT, 128], mdt, kind="ExternalInput")
    fhi = nc.dram_tensor("fhi", [NPAD - SPLIT, 128], mdt, kind="ExternalInput")
    fshard = nc.dram_tensor("fshard", [SHARD, D], f32, kind="ExternalInput")
    finit = nc.dram_tensor("finit", [SHARD, D], f32, kind="ExternalInput")
    gidx_d = nc.dram_tensor("gidx", [128, GCOLS], i16, kind="ExternalInput")
    meta_d = nc.dram_tensor("meta", [128, MCOLS], f32, kind="ExternalInput")
    # bf16 metadata: [iota_rep | dstloc] (both integer-valued, bf16-exact)
    metab_d = nc.dram_tensor(
        "metab", [128, W * SGMAX + TOT], bf16, kind="ExternalInput"
    )
    out_d = nc.dram_tensor("out", [SHARD, D], f32, kind="ExternalOutput")

    with tile.TileContext(nc) as tc:
        with (
            tc.tile_pool(name="const", bufs=1) as cpool,
            tc.tile_pool(name="gath", bufs=2) as gpool,
            tc.tile_pool(name="oh", bufs=2) as opool,
            tc.tile_pool(name="fin", bufs=2) as fpool,
            tc.tile_pool(name="psum", bufs=4, space="PSUM") as ppool,
        ):
            # ---- constant/metadata loads ----
            gidx_t = cpool.tile([128, GCOLS], i16)
            nc.sync.dma_start(out=gidx_t[:], in_=gidx_d[:, :])
            meta_t = cpool.tile([128, MCOLS], f32)
            nc.sync.dma_start(out=meta_t[:], in_=meta_d[:, :])
            dstloc_t = meta_t[:, 0:TOT]
            degsrc_t = meta_t[:, TOT : 2 * TOT]
            c0 = 2 * TOT
            degdst_t = meta_t[:, c0 : c0 + NPAIRS]
            iota_t = meta_t[:, c0 + NPAIRS : c0 + NPAIRS + W]
            awb = meta_t[:, c0 + NPAIRS + W : c0 + NPAIRS + W + 2 * D]

            metab_t = cpool.tile([128, W * SGMAX + TOT], bf16)
            nc.sync.dma_start(out=metab_t[:], in_=metab_d[:, :])
            iota_rep = metab_t[:, 0 : W * SGMAX]  # [128, W, SGMAX] layout
            dstloc_b = metab_t[:, W * SGMAX :]

            # norm = 1/sqrt(deg) (deg pre-clamped >=1 host-side, integer op)
            nc.scalar.sqrt(out=degsrc_t, in_=degsrc_t)
            nc.vector.reciprocal(out=degsrc_t, in_=degsrc_t)
            nc.scalar.sqrt(out=degdst_t, in_=degdst_t)
            nc.vector.reciprocal(out=degdst_t, in_=degdst_t)
            # norm_src in message dtype for the one-hot weighting
            normsrc_m = cpool.tile([128, TOT], mdt)
            nc.vector.tensor_copy(out=normsrc_m[:], in_=degsrc_t)
            dstloc_m = dstloc_b if USE_BF16 else dstloc_t
            if USE_BF16:
                iota_m = iota_rep
            else:
                iota_m = None  # f32 path uses broadcast iota from meta

            h_all = cpool.tile([128, NPAIRS, D], f32)

            # ---- main scatter loop over gather groups ----
            _ng = int(_os.environ.get("K_NGROUPS", len(groups)))
            for g, bs in enumerate(groups[:_ng]):
                lo0, lon = call_info[g]["lo"]
                hi0, hin = call_info[g]["hi"]
                sg0, sgn = lo0, lon + hin
                col0 = call_cols[g][0]

                gath = gpool.tile([128, SGMAX, 128], mdt, tag="gath")
                # one-hot stored [128, W, slot] so every DVE operand keeps
                # innermost stride 1 (2x_1p eligibility in bf16)
                oh = opool.tile([128, W, SGMAX], mdt, tag="oh")

                nc.gpsimd.dma_gather(
                    gath[:, 0:lon, :],
                    flo[:, :],
                    gidx_t[:, col0 : col0 + lon * 8],
                    lon * 128,
                    lon * 128,
                    128,
                    elem_step=128,
                    single_packet=False,
                )
                nc.gpsimd.dma_gather(
                    gath[:, lon : lon + hin, :],
                    fhi[:, :],
                    gidx_t[:, col0 + lon * 8 : col0 + (lon + hin) * 8],
                    hin * 128,
                    hin * 128,
                    128,
                    elem_step=128,
                    single_packet=False,
                )

                # weighted one-hot: oh[e, j, s] = (dstloc[e,s] == j) * norm_src[e,s]
                if USE_BF16:
                    in1 = iota_m[:].rearrange("p (j s) -> p j s", j=W)[:, :, 0:sgn]
                else:
                    in1 = iota_t[:, :, None].to_broadcast([128, W, sgn])
                nc.vector.tensor_tensor(
                    out=oh[:, :, 0:sgn],
                    in0=dstloc_m[:, None, sg0 : sg0 + sgn].to_broadcast(
                        [128, W, sgn]
                    ),
                    in1=in1,
                    op=OP.is_equal,
                )
                nc.vector.tensor_tensor(
                    out=oh[:, :, 0:sgn],
                    in0=oh[:, :, 0:sgn],
                    in1=normsrc_m[:, None, sg0 : sg0 + sgn].to_broadcast(
                        [128, W, sgn]
                    ),
                    op=OP.mult,
                )

                # scatter matmuls, accumulate per block into PSUM pair tiles
                for pi in range(0, len(bs), 2):
                    ptile = ppool.tile([128, D], f32, tag="ps", space="PSUM")
                    for j, b in enumerate(bs[pi : pi + 2]):
                        prange = ptile[j * W : (j + 1) * W, :]
                        mm_slots = []
                        for half in (0, 1):
                            seg = 2 * b + half
                            s0 = int(slot_start[seg]) - sg0
                            mm_slots += list(
                                range(s0, s0 + int(slots_of_seg[seg]))
                            )
                        for k, s in enumerate(mm_slots):
                            nc.tensor.matmul(
                                out=prange,
                                lhsT=oh[:, :, s],
                                rhs=gath[:, s, 0:D],
                                start=(k == 0),
                                stop=(k == len(mm_slots) - 1),
                                tile_position=(0, j * W),
                            )
                    pair = (bs[pi]) // 2
                    # h = psum * norm_dst  (fused into PSUM->SBUF copy)
                    nc.scalar.activation(
                        out=h_all[:, pair, :],
                        in_=ptile[:, :],
                        func=AF.Copy,
                        scale=degdst_t[:, pair : pair + 1],
                    )

            # ---- gate + output, chunked over pairs ----
            for k0 in range(0, NPAIRS, OUT_CH):
                k1 = min(k0 + OUT_CH, NPAIRS)
                kn = k1 - k0
                rows = slice(k0 * 128, k1 * 128)
                fch = fpool.tile([128, OUT_CH, D], f32, tag="fch")
                ich = fpool.tile([128, OUT_CH, D], f32, tag="ich")
                tmp = fpool.tile([128, OUT_CH, D], f32, tag="tmp")
                s1 = fpool.tile([128, OUT_CH], f32, tag="s1")
                s2 = fpool.tile([128, OUT_CH], f32, tag="s2")
                och = fpool.tile([128, OUT_CH, D], f32, tag="och")
                nc.sync.dma_start(
                    out=fch[:, 0:kn, :],
                    in_=fshard[rows, :].rearrange("(b p) f -> p b f", p=128),
                )
                nc.sync.dma_start(
                    out=ich[:, 0:kn, :],
                    in_=finit[rows, :].rearrange("(b p) f -> p b f", p=128),
                )
                # s1 = f . w1 ; s2 = h . w2
                nc.vector.tensor_tensor(
                    out=tmp[:, 0:kn, :],
                    in0=fch[:, 0:kn, :],
                    in1=awb[:, None, 0:D].to_broadcast([128, kn, D]),
                    op=OP.mult,
                )
                nc.vector.tensor_reduce(
                    out=s1[:, 0:kn],
                    in_=tmp[:, 0:kn, :],
                    axis=mybir.AxisListType.X,
                    op=OP.add,
                )
                nc.vector.tensor_tensor(
                    out=tmp[:, 0:kn, :],
                    in0=h_all[:, k0:k1, :],
                    in1=awb[:, None, D : 2 * D].to_broadcast([128, kn, D]),
                    op=OP.mult,
                )
                nc.vector.tensor_reduce(
                    out=s2[:, 0:kn],
                    in_=tmp[:, 0:kn, :],
                    axis=mybir.AxisListType.X,
                    op=OP.add,
                )
                nc.vector.tensor_add(out=s1[:, 0:kn], in0=s1[:, 0:kn], in1=s2[:, 0:kn])
                nc.scalar.activation(
                    out=s1[:, 0:kn], in_=s1[:, 0:kn], func=AF.Sigmoid
                )
                # out = alpha*h + init
                nc.vector.tensor_tensor(
                    out=och[:, 0:kn, :],
                    in0=h_all[:, k0:k1, :],
                    in1=s1[:, 0:kn, None].to_broadcast([128, kn, D]),
                    op=OP.mult,
                )
                nc.vector.tensor_add(
                    out=och[:, 0:kn, :], in0=och[:, 0:kn, :], in1=ich[:, 0:kn, :]
                )
                nc.sync.dma_start(
                    out=out_d[rows, :].rearrange("(b p) f -> p b f", p=128),
                    in_=och[:, 0:kn, :],
                )

    nc.finalize()
    return nc


def prepare(features, initial_features, a_weight, src, dst):
    features = np.asarray(features, dtype=np.float32)
    initial_features = np.asarray(initial_features, dtype=np.float32)
    a_weight = np.asarray(a_weight, dtype=np.float32)

    static, cores = _host_prep(src, dst)
    nc = _build_kernel(static)

    import os as _os
    import ml_dtypes

    use_bf16 = _os.environ.get("K_DT", "bf16") == "bf16"
    mdt_np = ml_dtypes.bfloat16 if use_bf16 else np.float32

    fpad = np.zeros((NPAD, 128), dtype=np.float32)
    fpad[:N, :D] = features
    init_pad = np.zeros((NPAD, D), dtype=np.float32)
    init_pad[:N] = initial_features
    flo_t = fpad[:SPLIT].astype(mdt_np)
    fhi_t = fpad[SPLIT:].astype(mdt_np)

    SGMAX = static["sgmax"]
    iota_rep = np.repeat(
        np.arange(W, dtype=np.float32), SGMAX
    )[None, :].astype(ml_dtypes.bfloat16)

    in_maps = []
    for c in range(NCORES):
        cc = cores[c]
        meta = cc["meta"]
        meta[:, meta.shape[1] - 2 * D :] = a_weight[0][None, :]
        metab = np.zeros((128, W * SGMAX + static["total_slots"]), dtype=ml_dtypes.bfloat16)
        metab[:, : W * SGMAX] = iota_rep
        metab[:, W * SGMAX :] = cc["dstloc"].astype(ml_dtypes.bfloat16)
        in_maps.append(
            dict(
                flo=flo_t,
                fhi=fhi_t,
                fshard=np.ascontiguousarray(
                    fpad[c * SHARD : (c + 1) * SHARD, :D]
                ),
                finit=np.ascontiguousarray(init_pad[c * SHARD : (c + 1) * SHARD]),
                gidx=cc["gidx"],
                meta=meta,
                metab=metab,
            )
        )
    return nc, in_maps


def kernel(features, initial_features, a_weight, src, dst):
    import concourse.bass_utils as bass_utils

    nc, in_maps = prepare(features, initial_features, a_weight, src, dst)

    global _last_nc, _last_in_maps
    _last_nc, _last_in_maps = nc, in_maps

    res = bass_utils.run_bass_kernel_spmd(nc, in_maps, core_ids=list(range(NCORES)))
    out = np.concatenate([r["out"] for r in res.results], axis=0)
    return out[:N]


_last_nc = None
_last_in_maps = None


# revision 35
# speedup vs baseline: 25.6915x; 25.6915x over previous
"""ASGC layer (gnn_message_passing) Trainium2 kernel.

Strategy: shard dst nodes 8 ways (graph-parallel per sharding hint). Host does
integer-only index preprocessing (edge bucketing by dst shard/block, slot
assignment, degree counting via bincount); all floating-point math runs on
device:
  - dma_gather fetches feature rows (padded to 512B) from HBM by src index
  - DVE builds norm[src]-weighted one-hot scatter matrices per 64-node block
  - PE matmuls accumulate segment sums into PSUM
  - ACT fuses the norm[dst] scale into the PSUM->SBUF copy
  - DVE/ACT compute the sigmoid gate and final output

src node ids exceed int16 gather-index range, so the padded feature table is
split into lo/hi halves at row 25088 and each block's edges are partitioned
into lo/hi slot groups (statically sized at max-over-cores).
"""

import numpy as np

N = 50000
D = 96
NPAD = 50176  # 392*128
NCORES = 8
SHARD = NPAD // NCORES  # 6272
W = 128  # dst nodes per scatter block (one-hot width)
BLOCKS = SHARD // W  # 49
BPG = 3  # blocks per gather group
NGROUPS = (BLOCKS + BPG - 1) // BPG
NPAIRS = SHARD // 128  # 49 [128,96] output tiles per core
SPLIT = 25088  # lo/hi gather table split
OUT_CH = 7  # pairs per output chunk (49 = 7*7)


def _cdiv(a, b):
    return (a + b - 1) // b


def _host_prep(src, dst):
    """Integer-only index preprocessing. Returns static schedule + per-core
    device input arrays."""
    src = np.asarray(src).astype(np.int64)
    dst = np.asarray(dst).astype(np.int64)
    deg = np.bincount(dst, minlength=NPAD).astype(np.int64)
    deg_cl = np.maximum(deg, 1).astype(np.float32)

    core_of_edge = dst // SHARD

    # per-core sorted edge arrays and per-seg counts; seg = block*2 + half
    NSEG = BLOCKS * 2
    per_core = []
    cnt = np.zeros((NCORES, NSEG), dtype=np.int64)
    for c in range(NCORES):
        m = core_of_edge == c
        s_c, d_c = src[m], dst[m]
        blk = (d_c - c * SHARD) // W
        half = (s_c >= SPLIT).astype(np.int64)
        seg = blk * 2 + half
        order = np.argsort(seg, kind="stable")
        s_c, d_c, seg = s_c[order], d_c[order], seg[order]
        cnt[c] = np.bincount(seg, minlength=NSEG)
        per_core.append((s_c, d_c, seg))

    # static slots per seg: max over cores, >= 1
    slots_of_seg = np.maximum(_cdiv(cnt.max(axis=0), 128), 1)  # ceil div
    slots_of_seg = np.maximum(
        (cnt.max(axis=0) + 127) // 128, 1
    ).astype(np.int64)

    groups = [
        list(range(g * BPG, min((g + 1) * BPG, BLOCKS))) for g in range(NGROUPS)
    ]

    # slot order: per group: [lo slots of each block, then hi slots of each block]
    slot_start = np.zeros(NSEG, dtype=np.int64)
    call_info = []  # per group: dict(lo=(slot0, nslots), hi=(...)) in slots
    cursor = 0
    for bs in groups:
        ginfo = {}
        for half in (0, 1):
            first = cursor
            for b in bs:
                seg = 2 * b + half
                slot_start[seg] = cursor
                cursor += int(slots_of_seg[seg])
            ginfo["lo" if half == 0 else "hi"] = (int(first), int(cursor - first))
        call_info.append(ginfo)
    total_slots = int(cursor)

    # gidx column layout: calls in order (g0 lo, g0 hi, g1 lo, ...), each call
    # with nslots*8 int16 columns
    col_cursor = 0
    call_cols = []
    for g in range(NGROUPS):
        lo0, lon = call_info[g]["lo"]
        hi0, hin = call_info[g]["hi"]
        call_cols.append((int(col_cursor), int(col_cursor + lon * 8)))
        col_cursor += (lon + hin) * 8
    gidx_cols = int(col_cursor)

    # per-core arrays
    cores = []
    for c in range(NCORES):
        s_c, d_c, seg = per_core[c]
        ne = len(s_c)
        seg_first = np.searchsorted(seg, np.arange(NSEG))
        rank = np.arange(ne) - seg_first[seg]
        slot = slot_start[seg] + rank // 128  # global slot column
        lane = rank % 128

        dstloc = np.full((128, total_slots), -1.0, dtype=np.float32)
        degsrc = np.ones((128, total_slots), dtype=np.float32)
        dstloc[lane, slot] = (d_c - (dst_base := c * SHARD) - (seg // 2) * W).astype(
            np.float32
        )
        degsrc[lane, slot] = deg_cl[s_c]

        # gather indices: position within call = (slot - call_slot0)*128 + lane
        gidx = np.zeros((128, gidx_cols), dtype=np.int16)
        idx_val = (s_c - (seg % 2) * SPLIT).astype(np.int16)
        # call id = group*2 + half; call slot0/col0 lookup per seg
        seg_g = (seg // 2) // BPG
        seg_half = seg % 2
        call_slot0 = np.zeros(NSEG, dtype=np.int64)
        call_col0 = np.zeros(NSEG, dtype=np.int64)
        for g in range(NGROUPS):
            lo0, lon = call_info[g]["lo"]
            hi0, hin = call_info[g]["hi"]
            c0 = call_cols[g][0]
            for b in groups[g]:
                call_slot0[2 * b] = lo0
                call_col0[2 * b] = c0
                call_slot0[2 * b + 1] = hi0
                call_col0[2 * b + 1] = c0 + lon * 8
        i_call = (slot - call_slot0[seg]) * 128 + lane
        col = call_col0[seg] + i_call // 16
        row = i_call % 16
        for rep in range(8):
            gidx[row + rep * 16, col] = idx_val

        degdst = deg_cl[c * SHARD : (c + 1) * SHARD].reshape(NPAIRS, 128).T

        # pack all f32 metadata into one tensor: one DMA -> one sem wait on
        # consumers (walrus caps sync waits per instruction)
        meta = np.zeros((128, 2 * total_slots + NPAIRS + W + 2 * D), dtype=np.float32)
        meta[:, :total_slots] = dstloc
        meta[:, total_slots : 2 * total_slots] = degsrc
        c0 = 2 * total_slots
        meta[:, c0 : c0 + NPAIRS] = degdst
        meta[:, c0 + NPAIRS : c0 + NPAIRS + W] = np.arange(W, dtype=np.float32)[
            None, :
        ]
        cores.append(dict(gidx=gidx, meta=meta, dstloc=dstloc))

    sgmax = max(
        call_info[g]["lo"][1] + call_info[g]["hi"][1] for g in range(NGROUPS)
    )
    # SBUF sizing bound: gather/one-hot tiles are [128, SGMAX, 128]. Uniform
    # random graphs give ~27 slots/group; extreme dst skew would need a
    # slot-budgeted grouping rewrite.
    assert sgmax <= 96, f"dst distribution too skewed for fixed grouping: {sgmax}"
    static = dict(
        slots_of_seg=slots_of_seg,
        slot_start=slot_start,
        groups=groups,
        call_info=call_info,
        call_cols=call_cols,
        total_slots=total_slots,
        gidx_cols=gidx_cols,
        sgmax=sgmax,
    )
    return static, cores


def _build_kernel(static):
    import concourse.bacc as bacc
    import concourse.bass as bass
    import concourse.mybir as mybir
    import concourse.tile as tile

    slots_of_seg = static["slots_of_seg"]
    slot_start = static["slot_start"]
    groups = static["groups"]
    call_info = static["call_info"]
    call_cols = static["call_cols"]
    TOT = static["total_slots"]
    GCOLS = static["gidx_cols"]
    SGMAX = max(
        call_info[g]["lo"][1] + call_info[g]["hi"][1] for g in range(NGROUPS)
    )

    f32 = mybir.dt.float32
    bf16 = mybir.dt.bfloat16
    i16 = mybir.dt.int16
    AF = mybir.ActivationFunctionType
    OP = mybir.AluOpType

    # message-passing path dtype (gather tables / gathered tiles / one-hots);
    # PSUM accumulation, degrees, gate and output stay fp32
    import os as _os

    USE_BF16 = _os.environ.get("K_DT", "bf16") == "bf16"
    mdt = bf16 if USE_BF16 else f32

    MCOLS = 2 * TOT + NPAIRS + W + 2 * D

    NQ = int(_os.environ.get("K_NQUEUES", "4"))
    nc = bacc.Bacc(None, target_bir_lowering=False, num_swdge_queues=NQ)
    flo = nc.dram_tensor("flo", [SPLIT, 128], mdt, kind="ExternalInput")
    fhi = nc.dram_tensor("fhi", [NPAD - SPLIT, 128], mdt, kind="ExternalInput")
    fshard = nc.dram_tensor("fshard", [SHARD, D], f32, kind="ExternalInput")
    finit = nc.dram_tensor("finit", [SHARD, D], f32, kind="ExternalInput")
    gidx_d = nc.dram_tensor("gidx", [128, GCOLS], i16, kind="ExternalInput")
    meta_d = nc.dram_tensor("meta", [128, MCOLS], f32, kind="ExternalInput")
    # bf16 metadata: [iota_rep | dstloc] (both integer-valued, bf16-exact)
    metab_d = nc.dram_tensor(
        "metab", [128, W * SGMAX + TOT], bf16, kind="ExternalInput"
    )
    out_d = nc.dram_tensor("out", [SHARD, D], f32, kind="ExternalOutput")

    with tile.TileContext(nc) as tc:
        with (
            tc.tile_pool(name="const", bufs=1) as cpool,
            tc.tile_pool(name="gath", bufs=4) as gpool,
            tc.tile_pool(name="oh", bufs=2) as opool,
            tc.tile_pool(name="fin", bufs=2) as fpool,
            tc.tile_pool(name="psum", bufs=4, space="PSUM") as ppool,
        ):
            # ---- constant/metadata loads ----
            gidx_t = cpool.tile([128, GCOLS], i16)
            nc.sync.dma_start(out=gidx_t[:], in_=gidx_d[:, :])
            meta_t = cpool.tile([128, MCOLS], f32)
            nc.sync.dma_start(out=meta_t[:], in_=meta_d[:, :])
            dstloc_t = meta_t[:, 0:TOT]
            degsrc_t = meta_t[:, TOT : 2 * TOT]
            c0 = 2 * TOT
            degdst_t = meta_t[:, c0 : c0 + NPAIRS]
            iota_t = meta_t[:, c0 + NPAIRS : c0 + NPAIRS + W]
            awb = meta_t[:, c0 + NPAIRS + W : c0 + NPAIRS + W + 2 * D]

            metab_t = cpool.tile([128, W * SGMAX + TOT], bf16)
            nc.sync.dma_start(out=metab_t[:], in_=metab_d[:, :])
            iota_rep = metab_t[:, 0 : W * SGMAX]  # [128, W, SGMAX] layout
            dstloc_b = metab_t[:, W * SGMAX :]

            # norm = 1/sqrt(deg) (deg pre-clamped >=1 host-side, integer op)
            nc.scalar.sqrt(out=degsrc_t, in_=degsrc_t)
            nc.vector.reciprocal(out=degsrc_t, in_=degsrc_t)
            nc.scalar.sqrt(out=degdst_t, in_=degdst_t)
            nc.vector.reciprocal(out=degdst_t, in_=degdst_t)
            # norm_src in message dtype for the one-hot weighting
            normsrc_m = cpool.tile([128, TOT], mdt)
            nc.vector.tensor_copy(out=normsrc_m[:], in_=degsrc_t)
            dstloc_m = dstloc_b if USE_BF16 else dstloc_t
            if USE_BF16:
                iota_m = iota_rep
            else:
                iota_m = None  # f32 path uses broadcast iota from meta

            h_all = cpool.tile([128, NPAIRS, D], f32)

            # ---- main scatter loop over gather groups ----
            _ng = int(_os.environ.get("K_NGROUPS", len(groups)))
            _nrep = int(_os.environ.get("K_REPEAT", "1"))
            _abl = _os.environ.get("K_ABLATE", "")
            if _abl:
                nc.gpsimd.memset(h_all[:], 0.0)
            for _rep, (g, bs) in enumerate(
                [(g, bs) for g, bs in enumerate(groups[:_ng])] * _nrep
            ):
                lo0, lon = call_info[g]["lo"]
                hi0, hin = call_info[g]["hi"]
                sg0, sgn = lo0, lon + hin
                col0 = call_cols[g][0]

                gath = gpool.tile([128, SGMAX, 128], mdt, tag="gath")
                # one-hot stored [128, W, slot] so every DVE operand keeps
                # innermost stride 1 (2x_1p eligibility in bf16)
                oh = opool.tile([128, W, SGMAX], mdt, tag="oh")

                if _abl in ("", "gather", "gathoh"):
                    nc.gpsimd.dma_gather(
                        gath[:, 0:lon, :],
                        flo[:, :],
                        gidx_t[:, col0 : col0 + lon * 8],
                        lon * 128,
                        lon * 128,
                        128,
                        elem_step=128,
                        single_packet=False,
                        queue_num=(2 * g) % NQ,
                    )
                    nc.gpsimd.dma_gather(
                        gath[:, lon : lon + hin, :],
                        fhi[:, :],
                        gidx_t[:, col0 + lon * 8 : col0 + (lon + hin) * 8],
                        hin * 128,
                        hin * 128,
                        128,
                        elem_step=128,
                        single_packet=False,
                        queue_num=(2 * g + 1) % NQ,
                    )

                # weighted one-hot: oh[e, j, s] = (dstloc[e,s] == j) * norm_src[e,s]
                if _abl in ("", "oh", "gathoh"):
                    if USE_BF16:
                        in1 = iota_m[:].rearrange("p (j s) -> p j s", j=W)[
                            :, :, 0:sgn
                        ]
                    else:
                        in1 = iota_t[:, :, None].to_broadcast([128, W, sgn])
                    nc.vector.tensor_tensor(
                        out=oh[:, :, 0:sgn],
                        in0=dstloc_m[:, None, sg0 : sg0 + sgn].to_broadcast(
                            [128, W, sgn]
                        ),
                        in1=in1,
                        op=OP.is_equal,
                    )
                    nc.vector.tensor_tensor(
                        out=oh[:, :, 0:sgn],
                        in0=oh[:, :, 0:sgn],
                        in1=normsrc_m[:, None, sg0 : sg0 + sgn].to_broadcast(
                            [128, W, sgn]
                        ),
                        op=OP.mult,
                    )

                # scatter matmuls, accumulate per block into its PSUM tile
                for b in bs if _abl == "" else []:
                    ptile = ppool.tile([128, D], f32, tag="ps", space="PSUM")
                    mm_slots = []
                    for half in (0, 1):
                        seg = 2 * b + half
                        s0 = int(slot_start[seg]) - sg0
                        mm_slots += list(range(s0, s0 + int(slots_of_seg[seg])))
                    for k, s in enumerate(mm_slots):
                        nc.tensor.matmul(
                            out=ptile[:, :],
                            lhsT=oh[:, :, s],
                            rhs=gath[:, s, 0:D],
                            start=(k == 0),
                            stop=(k == len(mm_slots) - 1),
                        )
                    # h = psum * norm_dst  (fused into PSUM->SBUF copy)
                    nc.scalar.activation(
                        out=h_all[:, b, :],
                        in_=ptile[:, :],
                        func=AF.Copy,
                        scale=degdst_t[:, b : b + 1],
                    )

            # ---- gate + output, chunked over pairs ----
            for k0 in range(0, NPAIRS, OUT_CH):
                k1 = min(k0 + OUT_CH, NPAIRS)
                kn = k1 - k0
                rows = slice(k0 * 128, k1 * 128)
                fch = fpool.tile([128, OUT_CH, D], f32, tag="fch")
                ich = fpool.tile([128, OUT_CH, D], f32, tag="ich")
                tmp = fpool.tile([128, OUT_CH, D], f32, tag="tmp")
                s1 = fpool.tile([128, OUT_CH], f32, tag="s1")
                s2 = fpool.tile([128, OUT_CH], f32, tag="s2")
                och = fpool.tile([128, OUT_CH, D], f32, tag="och")
                nc.sync.dma_start(
                    out=fch[:, 0:kn, :],
                    in_=fshard[rows, :].rearrange("(b p) f -> p b f", p=128),
                )
                nc.sync.dma_start(
                    out=ich[:, 0:kn, :],
                    in_=finit[rows, :].rearrange("(b p) f -> p b f", p=128),
                )
                # s1 = f . w1 ; s2 = h . w2
                nc.vector.tensor_tensor(
                    out=tmp[:, 0:kn, :],
                    in0=fch[:, 0:kn, :],
                    in1=awb[:, None, 0:D].to_broadcast([128, kn, D]),
                    op=OP.mult,
                )
                nc.vector.tensor_reduce(
                    out=s1[:, 0:kn],
                    in_=tmp[:, 0:kn, :],
                    axis=mybir.AxisListType.X,
                    op=OP.add,
                )
                nc.vector.tensor_tensor(
                    out=tmp[:, 0:kn, :],
                    in0=h_all[:, k0:k1, :],
                    in1=awb[:, None, D : 2 * D].to_broadcast([128, kn, D]),
                    op=OP.mult,
                )
                nc.vector.tensor_reduce(
                    out=s2[:, 0:kn],
                    in_=tmp[:, 0:kn, :],
                    axis=mybir.AxisListType.X,
                    op=OP.add,
                )
                nc.vector.tensor_add(out=s1[:, 0:kn], in0=s1[:, 0:kn], in1=s2[:, 0:kn])
                nc.scalar.activation(
                    out=s1[:, 0:kn], in_=s1[:, 0:kn], func=AF.Sigmoid
                )
                # out = alpha*h + init
                nc.vector.tensor_tensor(
                    out=och[:, 0:kn, :],
                    in0=h_all[:, k0:k1, :],
                    in1=s1[:, 0:kn, None].to_broadcast([128, kn, D]),
                    op=OP.mult,
                )
                nc.vector.tensor_add(
                    out=och[:, 0:kn, :], in0=och[:, 0:kn, :], in1=ich[:, 0:kn, :]
                )
                nc.sync.dma_start(
                    out=out_d[rows, :].rearrange("(b p) f -> p b f", p=128),
                    in_=och[:, 0:kn, :],
                )

    nc.finalize()
    return nc


def prepare(features, initial_features, a_weight, src, dst):
    features = np.asarray(features, dtype=np.float32)
    initial_features = np.asarray(initial_features, dtype=np.float32)
    a_weight = np.asarray(a_weight, dtype=np.float32)

    static, cores = _host_prep(src, dst)
    nc = _build_kernel(static)

    import os as _os
    import ml_dtypes

    use_bf16 = _os.environ.get("K_DT", "bf16") == "bf16"
    mdt_np = ml_dtypes.bfloat16 if use_bf16 else np.float32

    fpad = np.zeros((NPAD, 128), dtype=np.float32)
    fpad[:N, :D] = features
    init_pad = np.zeros((NPAD, D), dtype=np.float32)
    init_pad[:N] = initial_features
    flo_t = fpad[:SPLIT].astype(mdt_np)
    fhi_t = fpad[SPLIT:].astype(mdt_np)

    SGMAX = static["sgmax"]
    iota_rep = np.repeat(
        np.arange(W, dtype=np.float32), SGMAX
    )[None, :].astype(ml_dtypes.bfloat16)

    in_maps = []
    for c in range(NCORES):
        cc = cores[c]
        meta = cc["meta"]
        meta[:, meta.shape[1] - 2 * D :] = a_weight[0][None, :]
        metab = np.zeros((128, W * SGMAX + static["total_slots"]), dtype=ml_dtypes.bfloat16)
        metab[:, : W * SGMAX] = iota_rep
        metab[:, W * SGMAX :] = cc["dstloc"].astype(ml_dtypes.bfloat16)
        in_maps.append(
            dict(
                flo=flo_t,
                fhi=fhi_t,
                fshard=np.ascontiguousarray(
                    fpad[c * SHARD : (c + 1) * SHARD, :D]
                ),
                finit=np.ascontiguousarray(init_pad[c * SHARD : (c + 1) * SHARD]),
                gidx=cc["gidx"],
                meta=meta,
                metab=metab,
            )
        )
    return nc, in_maps


def kernel(features, initial_features, a_weight, src, dst):
    import concourse.bass_utils as bass_utils

    nc, in_maps = prepare(features, initial_features, a_weight, src, dst)

    global _last_nc, _last_in_maps
    _last_nc, _last_in_maps = nc, in_maps

    res = bass_utils.run_bass_kernel_spmd(nc, in_maps, core_ids=list(range(NCORES)))
    out = np.concatenate([r["out"] for r in res.results], axis=0)
    return out[:N]


_last_nc = None
_last_in_maps = None
